# revision 5
# baseline (speedup 1.0000x reference)
"""Bass/Trainium2 kernel for nn_GATModel (hetero 2-layer GAT, 8 relations).

Sharding: relation r -> NeuronCore r (8 relations, 8 cores). One launch per
layer; the same compiled program serves both layers (dims match: IN = D = 128).

Device, per relation per layer:
  - projection hs = x_si @ Ws           (PE; bf16 in, fp32 PSUM, bf16 out)
  - hs written to HBM [N,128] bf16 rows (256B, dma_gather-able)
  - dma_gather of hs rows by edge src   (4 src-chunk streams, int16 local idx)
  - S0[e,j] = (dst_local[e] == j)       (DVE is_equal vs iota, per 128-edge tile)
  - msg = hs_gathered * alpha           (DVE; alpha precomputed on host)
  - agg[window] += S0.T @ msg           (PE, fp32 PSUM accumulation over the
                                         4 chunk tiles of each 128-dst window;
                                         each output row written exactly once
                                         -> no scatter, no RMW races)

Host (cheap glue, O(E*H) / O(N*D) numpy):
  - attention coefficients alpha = softmax_dst(leakyrelu(es[src]+ed[dst]))
    in exact fp32 (es/ed are [N,4] linear heads folded from Ws/Wd and a_s/a_d)
  - edge bucketing into (src-chunk, dst-window) tiles of 128 slots; the ~1e-3
    fraction of window-overflow edges is patched on host
  - cross-relation sums + bias + ELU between layers and at the end

Self-contained: shapes hardcoded; no sibling imports.
"""
import numpy as np
import ml_dtypes

BF16 = ml_dtypes.bfloat16

N = 100000
IN = 128
H = 4
HC = 32
D = H * HC          # 128
R = 8
E = 300000
REL = [(0, 1), (1, 0), (0, 2), (2, 0), (0, 3), (3, 0), (0, 4), (4, 0)]

SC = 4              # src chunks (int16 gather index range)
CS = -(-N // SC)    # 25000 rows per chunk
NW = -(-N // 128)   # 782 dst windows
SLOTS = NW * 128    # per chunk stream
TOT = SC * SLOTS    # 400384 edge slots
GROUP_W = 8         # windows per gather call (1024 indices/call)

_CACHE = {}


# ---------------- device program ----------------

def _build_program():
    import concourse.bacc as bacc
    import concourse.mybir as mybir
    import concourse.tile as tile

    nc = bacc.Bacc("TRN2", target_bir_lowering=False, debug=False,
                   enable_asserts=False)
    bf = mybir.dt.bfloat16
    f32 = mybir.dt.float32
    i16 = mybir.dt.int16

    xT_t = nc.dram_tensor("xT", [128, N], bf, kind="ExternalInput")
    w_t = nc.dram_tensor("w", [128, D], bf, kind="ExternalInput")
    iota_t = nc.dram_tensor("iota", [128, 128], bf, kind="ExternalInput")
    gidx_t = nc.dram_tensor("gidx", [128, TOT // 16], i16, kind="ExternalInput")
    dr_t = nc.dram_tensor("dstrel", [128, NW, SC], bf, kind="ExternalInput")
    al_t = nc.dram_tensor("alpha", [128, NW, SC * H], bf, kind="ExternalInput")
    hs_t = nc.dram_tensor("hs", [N, D], bf, kind="Internal")
    agg_t = nc.dram_tensor("agg", [N, D], f32, kind="ExternalOutput")

    with tile.TileContext(nc) as tc:
        with tc.tile_pool(name="stat", bufs=1) as spool, \
             tc.tile_pool(name="proj", bufs=4) as ppool, \
             tc.tile_pool(name="ps", bufs=3, space="PSUM") as pspool, \
             tc.tile_pool(name="gb", bufs=3) as gpool, \
             tc.tile_pool(name="ed", bufs=4) as epool:

            w_sb = spool.tile([128, D], bf)
            nc.sync.dma_start(out=w_sb[:], in_=w_t.ap())
            iota_sb = spool.tile([128, 128], bf)
            nc.sync.dma_start(out=iota_sb[:], in_=iota_t.ap())
            gidx_sb = spool.tile([128, TOT // 16], i16)
            nc.sync.dma_start(out=gidx_sb[:], in_=gidx_t.ap())
            dr_sb = spool.tile([128, NW, SC], bf)
            nc.sync.dma_start(out=dr_sb[:], in_=dr_t.ap())
            al_sb = spool.tile([128, NW, SC * H], bf)
            nc.sync.dma_start(out=al_sb[:], in_=al_t.ap())

            # phase 1: projection, node-major bf16 hs table
            for t in range(NW):
                lo = t * 128
                wd = min(128, N - lo)
                xt = ppool.tile([128, 128], bf, tag="x")
                nc.sync.dma_start(out=xt[:, :wd], in_=xT_t.ap()[:, lo:lo + wd])
                ps = pspool.tile([128, D], f32, tag="mm")
                nc.tensor.matmul(ps[:wd, :], xt[:, :wd], w_sb[:],
                                 start=True, stop=True)
                ht = ppool.tile([128, D], bf, tag="h")
                nc.scalar.activation(ht[:wd, :], ps[:wd, :],
                                     mybir.ActivationFunctionType.Copy)
                nc.sync.dma_start(out=hs_t.ap()[lo:lo + wd, :], in_=ht[:wd, :])

            tc.strict_bb_all_engine_barrier()

            # phase 2: gather + window aggregation
            ngroups = -(-NW // GROUP_W)
            for g in range(ngroups):
                w0 = g * GROUP_W
                gw = min(GROUP_W, NW - w0)
                gb = gpool.tile([128, SC, GROUP_W, D], bf, tag="gb")
                for c in range(SC):
                    slot0 = c * SLOTS + w0 * 128
                    nidx = gw * 128
                    rows = min(CS, N - c * CS)
                    nc.gpsimd.dma_gather(
                        out_ap=gb[:, c, :gw, :],
                        in_ap=hs_t.ap()[c * CS:c * CS + rows, :],
                        idxs_ap=gidx_sb[:, slot0 // 16:(slot0 + nidx) // 16],
                        num_idxs=nidx,
                        num_idxs_reg=nidx,
                        elem_size=D,
                    )
                for wi in range(gw):
                    w = w0 + wi
                    wrows = min(128, N - w * 128)
                    s0 = epool.tile([128, SC, 128], bf, tag="s0")
                    nc.vector.tensor_tensor(
                        out=s0[:],
                        in0=dr_sb[:, w, :, None].to_broadcast([128, SC, 128]),
                        in1=iota_sb[:, None, :].to_broadcast([128, SC, 128]),
                        op=mybir.AluOpType.is_equal,
                    )
                    msg = epool.tile([128, SC, D], bf, tag="msg")
                    for h in range(H):
                        nc.vector.tensor_tensor(
                            out=msg[:, :, h * HC:(h + 1) * HC],
                            in0=gb[:, :, wi, h * HC:(h + 1) * HC],
                            in1=al_sb[:, w, h::H, None].to_broadcast([128, SC, HC]),
                            op=mybir.AluOpType.mult,
                        )
                    ps = pspool.tile([128, D], f32, tag="agg")
                    for c in range(SC):
                        nc.tensor.matmul(ps[:], s0[:, c, :], msg[:, c, :],
                                         start=(c == 0), stop=(c == SC - 1))
                    ot = epool.tile([128, D], f32, tag="out")
                    nc.scalar.activation(ot[:wrows, :], ps[:wrows, :],
                                         mybir.ActivationFunctionType.Copy)
                    nc.sync.dma_start(out=agg_t.ap()[w * 128:w * 128 + wrows, :],
                                      in_=ot[:wrows, :])
    nc.compile()
    return nc


# ---------------- host-side helpers ----------------

def _bucketize(src, dst):
    """(chunk, window)-tile the edges. 128 slots per tile, overflow -> host."""
    c = src // CS
    w = dst // 128
    key = c * NW + w
    order = np.argsort(key, kind="stable")
    ks = key[order]
    n = src.shape[0]
    new = np.ones(n, bool)
    new[1:] = ks[1:] != ks[:-1]
    run_id = np.cumsum(new) - 1
    run_start = np.flatnonzero(new)
    rank = np.arange(n) - run_start[run_id]
    keep = rank < 128
    slot = ks * 128 + rank
    slot_edge = np.full(TOT, -1, np.int64)
    slot_edge[slot[keep]] = order[keep]
    overflow = order[~keep]

    gidx = np.zeros(TOT, np.int16)
    filled = slot_edge >= 0
    cslot = np.arange(TOT) // SLOTS
    gidx[filled] = (src[slot_edge[filled]] - cslot[filled] * CS).astype(np.int16)
    g16 = gidx.reshape(-1, 16).T
    gidx_w = np.ascontiguousarray(np.tile(g16, (8, 1)))

    wslot = (np.arange(TOT) % SLOTS) // 128
    dr = np.full(TOT, -1.0, np.float32)
    dr[filled] = (dst[slot_edge[filled]] - wslot[filled] * 128).astype(np.float32)
    dr3 = np.ascontiguousarray(
        dr.reshape(SC, NW, 128).transpose(2, 1, 0).astype(BF16))
    return gidx_w, dr3, slot_edge, overflow


def _slot_alpha(alpha, slot_edge):
    al = np.zeros((TOT, H), np.float32)
    filled = slot_edge >= 0
    al[filled] = alpha[slot_edge[filled]]
    al4 = al.reshape(SC, NW, 128, H).transpose(2, 1, 0, 3)  # [i, w, c, h]
    return np.ascontiguousarray(al4.reshape(128, NW, SC * H).astype(BF16))


def _fold_head(Wfull, a):
    """[128,128] weight + [H,HC] head vector -> [128,H] so x@out == einsum."""
    M = np.zeros((D, H), np.float32)
    for h in range(H):
        M[h * HC:(h + 1) * HC, h] = a[h]
    return (Wfull @ M).astype(np.float32)


def _host_alpha(x_src, x_dst, Ws, a_s, Wd, a_d, src, dst):
    """Exact fp32 softmax attention coefficients (shift-invariant form)."""
    es = x_src @ _fold_head(Ws, a_s)   # [N, H]
    ed = x_dst @ _fold_head(Wd, a_d)
    z = es[src] + ed[dst]
    logit = np.where(z > 0, z, np.float32(0.2) * z).astype(np.float32)
    p = np.exp(logit)
    denom = np.empty((N, H), np.float32)
    for h in range(H):
        denom[:, h] = np.bincount(dst, weights=p[:, h], minlength=N)
    return (p / (denom[dst] + 1e-16)).astype(np.float32)


def _elu(x):
    return np.where(x > 0, x, np.expm1(np.minimum(x, 0.0))).astype(np.float32)


def _run_layer(nc, xs, edges, Ws, Wd, a_s, a_d, b, prep, iota):
    from concourse import bass_utils
    in_maps = []
    alphas = []
    for r, (si, di) in enumerate(REL):
        gidx_w, dr3, slot_edge, _overflow = prep[r]
        src = edges[r, 0]
        dst = edges[r, 1]
        alpha = _host_alpha(xs[si], xs[di], Ws[r], a_s[r], Wd[r], a_d[r],
                            src, dst)
        alphas.append(alpha)
        in_maps.append({
            "xT": np.ascontiguousarray(xs[si].T).astype(BF16),
            "w": Ws[r].astype(BF16),
            "iota": iota,
            "gidx": gidx_w,
            "dstrel": dr3,
            "alpha": _slot_alpha(alpha, slot_edge),
        })
    res = bass_utils.run_bass_kernel_spmd(nc, in_maps, core_ids=list(range(8)))
    outs = [np.zeros((N, D), np.float32) for _ in range(5)]
    for r, (si, di) in enumerate(REL):
        agg = np.array(res.results[r]["agg"], np.float32)  # writable copy
        overflow = prep[r][3]
        if overflow.size:
            src = edges[r, 0][overflow]
            dst = edges[r, 1][overflow]
            hs_b = (xs[si].astype(BF16).astype(np.float32)
                    @ Ws[r].astype(BF16).astype(np.float32)).astype(BF16).astype(np.float32)
            msg = hs_b[src] * np.repeat(alphas[r][overflow], HC, axis=1)
            np.add.at(agg, dst, msg.astype(np.float32))
        outs[di] += agg + b[r]
    return [_elu(o) for o in outs]


def kernel(x_transaction, x_account, x_device, x_ip, x_email, edges,
           Ws1, Wd1, as1, ad1, b1, Ws2, Wd2, as2, ad2, b2):
    xs = [np.asarray(x, np.float32) for x in
          (x_transaction, x_account, x_device, x_ip, x_email)]
    edges = np.asarray(edges).astype(np.int64)
    Ws1, Wd1 = np.asarray(Ws1, np.float32), np.asarray(Wd1, np.float32)
    Ws2, Wd2 = np.asarray(Ws2, np.float32), np.asarray(Wd2, np.float32)
    as1, ad1 = np.asarray(as1, np.float32), np.asarray(ad1, np.float32)
    as2, ad2 = np.asarray(as2, np.float32), np.asarray(ad2, np.float32)
    b1, b2 = np.asarray(b1, np.float32), np.asarray(b2, np.float32)

    prep = [_bucketize(edges[r, 0], edges[r, 1]) for r in range(R)]
    iota = np.tile(np.arange(128, dtype=np.float32), (128, 1)).astype(BF16)

    if "nc" not in _CACHE:
        _CACHE["nc"] = _build_program()
    nc = _CACHE["nc"]

    xs = _run_layer(nc, xs, edges, Ws1, Wd1, as1, ad1, b1, prep, iota)
    xs = _run_layer(nc, xs, edges, Ws2, Wd2, as2, ad2, b2, prep, iota)
    return np.stack(xs).astype(np.float32)


# revision 13
# speedup vs baseline: 2.0273x; 2.0273x over previous
"""Bass/Trainium2 kernel for nn_GATModel (hetero 2-layer GAT, 8 relations).

Sharding: relation r -> NeuronCore r (8 relations, 8 cores). One launch per
layer; the same compiled program serves both layers (dims match: IN = D = 128).

Device, per relation per layer:
  - projection hs = x_si @ Ws           (PE; bf16 in, fp32 PSUM, bf16 out)
  - hs written to HBM [N,128] bf16 rows (256B, dma_gather-able)
  - dma_gather of hs rows by edge src   (4 src-chunk streams, int16 local idx)
  - S0[e,j] = (dst_local[e] == j)       (DVE is_equal vs iota, per 128-edge tile)
  - msg = hs_gathered * alpha           (DVE; alpha precomputed on host)
  - agg[window] += S0.T @ msg           (PE, fp32 PSUM accumulation over the
                                         4 chunk tiles of each 128-dst window;
                                         each output row written exactly once
                                         -> no scatter, no RMW races)

Host (cheap glue, O(E*H) / O(N*D) numpy):
  - attention coefficients alpha = softmax_dst(leakyrelu(es[src]+ed[dst]))
    in exact fp32 (es/ed are [N,4] linear heads folded from Ws/Wd and a_s/a_d)
  - edge bucketing into (src-chunk, dst-window) tiles of 128 slots; the ~1e-3
    fraction of window-overflow edges is patched on host
  - cross-relation sums + bias + ELU between layers and at the end

Self-contained: shapes hardcoded; no sibling imports.
"""
import numpy as np
import ml_dtypes

BF16 = ml_dtypes.bfloat16

N = 100000
IN = 128
H = 4
HC = 32
D = H * HC          # 128
R = 8
E = 300000
REL = [(0, 1), (1, 0), (0, 2), (2, 0), (0, 3), (3, 0), (0, 4), (4, 0)]

NQ = 4              # SWDGE queues (parallel Q7 desc-gen: ~6x faster gathers)
SC = 4              # src chunks (int16 gather index range)
CS = -(-N // SC)    # 25000 rows per chunk
NW = -(-N // 128)   # 782 dst windows
SLOTS = NW * 128    # per chunk stream
TOT = SC * SLOTS    # 400384 edge slots
GROUP_W = 8         # windows per gather call (1024 indices/call)

_CACHE = {}


# ---------------- device program ----------------

def _build_program():
    import concourse.bacc as bacc
    import concourse.mybir as mybir
    import concourse.tile as tile

    nc = bacc.Bacc("TRN2", target_bir_lowering=False, debug=False,
                   enable_asserts=False, num_swdge_queues=NQ)
    bf = mybir.dt.bfloat16
    f32 = mybir.dt.float32
    i16 = mybir.dt.int16

    tok_t = nc.dram_tensor("tok", [128, 128], f32, kind="ExternalInput")
    tokout_t = nc.dram_tensor("tokout", [128, 128], f32, kind="ExternalOutput")
    xT_t = nc.dram_tensor("xT", [128, N], bf, kind="ExternalInput")
    w_t = nc.dram_tensor("w", [128, D], bf, kind="ExternalInput")
    iota_t = nc.dram_tensor("iota", [128, 128], bf, kind="ExternalInput")
    gidx_t = nc.dram_tensor("gidx", [128, TOT // 16], i16, kind="ExternalInput")
    dr_t = nc.dram_tensor("dstrel", [128, NW, SC], bf, kind="ExternalInput")
    al_t = nc.dram_tensor("alpha", [128, NW, SC * H], bf, kind="ExternalInput")
    hs_t = nc.dram_tensor("hs", [N, D], bf, kind="Internal")
    agg_t = nc.dram_tensor("agg", [N, D], f32, kind="ExternalOutput")

    GW = GROUP_W
    NG = -(-NW // GW)

    with tile.TileContext(nc) as tc:
        with tc.tile_pool(name="stat", bufs=1) as spool, \
             tc.tile_pool(name="proj", bufs=3) as ppool, \
             tc.tile_pool(name="psm", bufs=2, space="PSUM") as psm, \
             tc.tile_pool(name="psa", bufs=3, space="PSUM") as psa, \
             tc.tile_pool(name="gb", bufs=3) as gpool, \
             tc.tile_pool(name="ed", bufs=4) as epool:

            tk = spool.tile([128, 128], f32)
            nc.sync.dma_start(out=tk[:], in_=tok_t.ap())
            nc.sync.dma_start(out=tokout_t.ap(), in_=tk[:])
            w_sb = spool.tile([128, D], bf)
            nc.sync.dma_start(out=w_sb[:], in_=w_t.ap())
            iota_sb = spool.tile([128, 128], bf)
            nc.sync.dma_start(out=iota_sb[:], in_=iota_t.ap())
            gidx_sb = spool.tile([128, TOT // 16], i16)
            nc.sync.dma_start(out=gidx_sb[:], in_=gidx_t.ap())
            dr_sb = spool.tile([128, NW, SC], bf)
            nc.sync.dma_start(out=dr_sb[:], in_=dr_t.ap())
            al_sb = spool.tile([128, NW, SC * H], bf)
            nc.sync.dma_start(out=al_sb[:], in_=al_t.ap())

            # phase 1: projection in groups of GW node-tiles (one DMA in/out)
            for g in range(NG):
                lo = g * GW * 128
                nt = min(GW * 128, N - lo)           # nodes in group
                ntiles = -(-nt // 128)
                xt = ppool.tile([128, GW * 128], bf, tag="x")
                nc.sync.dma_start(out=xt[:, :nt], in_=xT_t.ap()[:, lo:lo + nt])
                ps = psm.tile([128, GW, D], f32, tag="mm")
                for t in range(ntiles):
                    wd = min(128, nt - t * 128)
                    nc.tensor.matmul(ps[:wd, t, :],
                                     xt[:, t * 128:t * 128 + wd], w_sb[:],
                                     start=True, stop=True)
                ht = ppool.tile([128, GW, D], bf, tag="h")
                nc.scalar.activation(ht[:, :ntiles, :], ps[:, :ntiles, :],
                                     mybir.ActivationFunctionType.Copy)
                # node-major store: node = lo + t*128 + p
                nfull = nt // 128
                if nfull:
                    dst = hs_t.ap()[lo:lo + nfull * 128, :] \
                        .rearrange("(t p) f -> p t f", p=128)
                    nc.sync.dma_start(out=dst, in_=ht[:, :nfull, :])
                if nt % 128:
                    wd = nt % 128
                    nc.sync.dma_start(
                        out=hs_t.ap()[lo + nfull * 128:lo + nt, :],
                        in_=ht[:wd, nfull, :])

            tc.strict_bb_all_engine_barrier()

            # phase 2: gather + window aggregation
            for g in range(NG):
                w0 = g * GW
                gw = min(GW, NW - w0)
                gb = gpool.tile([128, SC, GW, D], bf, tag="gb")
                for c in range(SC):
                    slot0 = c * SLOTS + w0 * 128
                    nidx = gw * 128
                    rows = min(CS, N - c * CS)
                    nc.gpsimd.dma_gather(
                        out_ap=gb[:, c, :gw, :],
                        in_ap=hs_t.ap()[c * CS:c * CS + rows, :],
                        idxs_ap=gidx_sb[:, slot0 // 16:(slot0 + nidx) // 16],
                        num_idxs=nidx,
                        num_idxs_reg=nidx,
                        elem_size=D,
                        queue_num=c % NQ,
                    )
                ob = epool.tile([128, GW, D], f32, tag="ob")
                for wi in range(gw):
                    w = w0 + wi
                    s0 = epool.tile([128, SC, 128], bf, tag="s0")
                    nc.vector.tensor_tensor(
                        out=s0[:],
                        in0=dr_sb[:, w, :, None].to_broadcast([128, SC, 128]),
                        in1=iota_sb[:, None, :].to_broadcast([128, SC, 128]),
                        op=mybir.AluOpType.is_equal,
                    )
                    msg = epool.tile([128, SC, D], bf, tag="msg")
                    nc.vector.tensor_tensor(
                        out=msg[:].rearrange("p c (h f) -> p c h f", f=HC),
                        in0=gb[:, :, wi, :].rearrange("p c (h f) -> p c h f", f=HC),
                        in1=al_sb[:, w, :, None]
                            .rearrange("p (c h) o -> p c h o", h=H)
                            .to_broadcast([128, SC, H, HC]),
                        op=mybir.AluOpType.mult,
                    )
                    ps = psa.tile([128, D], f32, tag="agg")
                    for c in range(SC):
                        nc.tensor.matmul(ps[:], s0[:, c, :], msg[:, c, :],
                                         start=(c == 0), stop=(c == SC - 1))
                    nc.scalar.activation(ob[:, wi, :], ps[:],
                                         mybir.ActivationFunctionType.Copy)
                lo = w0 * 128
                nt = min(gw * 128, N - lo)
                nfull = nt // 128
                if nfull:
                    dst = agg_t.ap()[lo:lo + nfull * 128, :] \
                        .rearrange("(t p) f -> p t f", p=128)
                    nc.sync.dma_start(out=dst, in_=ob[:, :nfull, :])
                if nt % 128:
                    wd = nt % 128
                    nc.sync.dma_start(
                        out=agg_t.ap()[lo + nfull * 128:lo + nt, :],
                        in_=ob[:wd, nfull, :])
    nc.compile()
    return nc


# ---------------- host-side helpers ----------------

def _bucketize(src, dst):
    """(chunk, window)-tile the edges. 128 slots per tile, overflow -> host."""
    c = src // CS
    w = dst // 128
    key = c * NW + w
    order = np.argsort(key, kind="stable")
    ks = key[order]
    n = src.shape[0]
    new = np.ones(n, bool)
    new[1:] = ks[1:] != ks[:-1]
    run_id = np.cumsum(new) - 1
    run_start = np.flatnonzero(new)
    rank = np.arange(n) - run_start[run_id]
    keep = rank < 128
    slot = ks * 128 + rank
    slot_edge = np.full(TOT, -1, np.int64)
    slot_edge[slot[keep]] = order[keep]
    overflow = order[~keep]

    gidx = np.zeros(TOT, np.int16)
    filled = slot_edge >= 0
    cslot = np.arange(TOT) // SLOTS
    gidx[filled] = (src[slot_edge[filled]] - cslot[filled] * CS).astype(np.int16)
    g16 = gidx.reshape(-1, 16).T
    gidx_w = np.ascontiguousarray(np.tile(g16, (8, 1)))

    wslot = (np.arange(TOT) % SLOTS) // 128
    dr = np.full(TOT, -1.0, np.float32)
    dr[filled] = (dst[slot_edge[filled]] - wslot[filled] * 128).astype(np.float32)
    dr3 = np.ascontiguousarray(
        dr.reshape(SC, NW, 128).transpose(2, 1, 0).astype(BF16))
    return gidx_w, dr3, slot_edge, overflow


def _slot_alpha(alpha, slot_edge):
    al = np.zeros((TOT, H), np.float32)
    filled = slot_edge >= 0
    al[filled] = alpha[slot_edge[filled]]
    al4 = al.reshape(SC, NW, 128, H).transpose(2, 1, 0, 3)  # [i, w, c, h]
    return np.ascontiguousarray(al4.reshape(128, NW, SC * H).astype(BF16))


def _fold_head(Wfull, a):
    """[128,128] weight + [H,HC] head vector -> [128,H] so x@out == einsum."""
    M = np.zeros((D, H), np.float32)
    for h in range(H):
        M[h * HC:(h + 1) * HC, h] = a[h]
    return (Wfull @ M).astype(np.float32)


def _host_alpha(x_src, x_dst, Ws, a_s, Wd, a_d, src, dst):
    """Exact fp32 softmax attention coefficients (shift-invariant form)."""
    es = x_src @ _fold_head(Ws, a_s)   # [N, H]
    ed = x_dst @ _fold_head(Wd, a_d)
    z = es[src] + ed[dst]
    logit = np.where(z > 0, z, np.float32(0.2) * z).astype(np.float32)
    p = np.exp(logit)
    denom = np.empty((N, H), np.float32)
    for h in range(H):
        denom[:, h] = np.bincount(dst, weights=p[:, h], minlength=N)
    return (p / (denom[dst] + 1e-16)).astype(np.float32)


def _elu(x):
    return np.where(x > 0, x, np.expm1(np.minimum(x, 0.0))).astype(np.float32)


def _run_layer(nc, xs, edges, Ws, Wd, a_s, a_d, b, prep, iota):
    from concourse import bass_utils
    in_maps = []
    alphas = []
    for r, (si, di) in enumerate(REL):
        gidx_w, dr3, slot_edge, _overflow = prep[r]
        src = edges[r, 0]
        dst = edges[r, 1]
        alpha = _host_alpha(xs[si], xs[di], Ws[r], a_s[r], Wd[r], a_d[r],
                            src, dst)
        alphas.append(alpha)
        in_maps.append({
            "tok": np.zeros((128, 128), np.float32),
            "xT": np.ascontiguousarray(xs[si].T).astype(BF16),
            "w": Ws[r].astype(BF16),
            "iota": iota,
            "gidx": gidx_w,
            "dstrel": dr3,
            "alpha": _slot_alpha(alpha, slot_edge),
        })
    res = bass_utils.run_bass_kernel_spmd(nc, in_maps, core_ids=list(range(8)))
    outs = [np.zeros((N, D), np.float32) for _ in range(5)]
    for r, (si, di) in enumerate(REL):
        agg = np.array(res.results[r]["agg"], np.float32)  # writable copy
        overflow = prep[r][3]
        if overflow.size:
            src = edges[r, 0][overflow]
            dst = edges[r, 1][overflow]
            hs_b = (xs[si].astype(BF16).astype(np.float32)
                    @ Ws[r].astype(BF16).astype(np.float32)).astype(BF16).astype(np.float32)
            msg = hs_b[src] * np.repeat(alphas[r][overflow], HC, axis=1)
            np.add.at(agg, dst, msg.astype(np.float32))
        outs[di] += agg + b[r]
    return [_elu(o) for o in outs]


def kernel(x_transaction, x_account, x_device, x_ip, x_email, edges,
           Ws1, Wd1, as1, ad1, b1, Ws2, Wd2, as2, ad2, b2):
    xs = [np.asarray(x, np.float32) for x in
          (x_transaction, x_account, x_device, x_ip, x_email)]
    edges = np.asarray(edges).astype(np.int64)
    Ws1, Wd1 = np.asarray(Ws1, np.float32), np.asarray(Wd1, np.float32)
    Ws2, Wd2 = np.asarray(Ws2, np.float32), np.asarray(Wd2, np.float32)
    as1, ad1 = np.asarray(as1, np.float32), np.asarray(ad1, np.float32)
    as2, ad2 = np.asarray(as2, np.float32), np.asarray(ad2, np.float32)
    b1, b2 = np.asarray(b1, np.float32), np.asarray(b2, np.float32)

    prep = [_bucketize(edges[r, 0], edges[r, 1]) for r in range(R)]
    iota = np.tile(np.arange(128, dtype=np.float32), (128, 1)).astype(BF16)

    if "nc" not in _CACHE:
        _CACHE["nc"] = _build_program()
    nc = _CACHE["nc"]

    xs = _run_layer(nc, xs, edges, Ws1, Wd1, as1, ad1, b1, prep, iota)
    xs = _run_layer(nc, xs, edges, Ws2, Wd2, as2, ad2, b2, prep, iota)
    return np.stack(xs).astype(np.float32)
